# revision 12
# baseline (speedup 1.0000x reference)
"""ArcFace margin loss (ArcMarginLoss) on 8 Trainium2 NeuronCores.

Classification-parallel sharding: V=32000 classes split across 8 cores
(4000 each, padded to 4096).  The device kernel is a pure fp8 GEMM +
exp-rowsum pipeline; everything O(B*D) or O(V*D)-elementwise lives on the
host.

Host prep (numpy):
  - x-hat = x/|x|, w-hat = W/|W| rows (fp32), scaled by 16 and cast to
    fp8 e4m3.  PSUM then holds 256*cos, and the exp stage applies scale
    s/256 and bias -s, computing exp(s*cos - 30) directly: since cos <= 1
    no global max pass or cross-core collective is needed.
  - Both operands are packed K-major for the PE's fp8 DoubleRow mode
    (2x MAC throughput).  Weights are packed piece-major
    ([P, piece, j, i, 512]) so each 512-class piece is one
    contiguous-per-partition DMA.
  - The label-column path (cos_y, phi, per-row exp corrections) is
    O(B*D), computed on the host in fp64.

Device per core (one SPMD NEFF), v3 pipeline:
  - Two phases of 2 class-quarters each.  Per (phase, m-tile): a single
    [128, 2048] fp32 PSUM tile (4 banks; 2-buf ring = whole PSUM) filled
    by 8 DoubleRow matmuls (4 x 512-col pieces x 2 k-planes).
  - Drains are split per m-tile between the two PSUM-capable engines
    (GpSimd has no PSUM port on TRN2):
      * 'S': ONE wide scalar ACT Exp over all 2048 cols with fused accum
        row-sum (amortizes the 352-cycle ACT ramp + 344ns accumulator
        read); in-place, and the 2-buf ring leaves 2072ns of drain
        latency slack so the 2344ns ACT fits without stalling the PE.
      * 'C': DVE tensor_scalar converts each 1024-col half of the s*cos
        logits straight to int16 bf16 BIT CODES (Schraudolph exp in the
        bf16 domain) into SBUF scratch -- the PSUM halves free after
        ~1.6us each -- then one fused tensor_scalar(mult 1, add 0) with
        accum_out over the codes read as bf16 yields the row-sum without
        a separate reduce pass.
    The (ph, m) -> engine map is tuned so both engines stay below the PE
    fill rate (~2072ns per m).
  - The first two m-tiles of phase 0 are walked q-major to hide the
    initial weight-DMA latency; DMA issues are split across the sync and
    scalar hardware-DGE queues so the critical pieces transfer in
    parallel.
  - HAM warm-up: real DoubleRow matmuls on a zeroed fp8 tile ramp the PE
    clock gate (transposes don't count as PE-busy for HAM).
  - Output: per-(phase, m) row sums Spart [128, 4, 16] fp32, DMA'd per
    phase.

Host epilogue: S = sum_c sum_q S_cq, scatter-add the label corrections,
loss = mean(30 + log(S) - s*phi_label).
"""

import math
import numpy as np
from contextlib import ExitStack

import concourse.bass as bass
import concourse.tile as tile
from concourse import bacc, mybir
from concourse import bass_utils
from concourse._compat import with_exitstack

P = 128
B = 2048          # batch rows
D = 512           # feature dim
V = 32000         # classes
NCORES = 8
VS = V // NCORES  # 4000 classes per core
VSP = 4096        # padded shard size
MT = B // P       # 16 batch row tiles
NJ = 2            # DoubleRow passes over D (each contracts 256)
NQ = 4            # class quarters per core
NPIECE = 8        # 512-class weight pieces per core
PW = (512, 512, 512, 512, 512, 512, 512, 416)  # real cols per piece

S_SCALE = 30.0
M_MARGIN = 0.5
SHIFT = 30.0      # exp(logit - SHIFT): logits <= 30 so always <= 0
WS = 16.0         # fp8 encode scale for x-hat and w-hat
EPS = 1e-12
ESC = S_SCALE / (WS * WS)   # psum -> logit scale (psum = 256*cos)

F32 = mybir.dt.float32
BF16 = mybir.dt.bfloat16
F8 = mybir.dt.float8e4
I16 = mybir.dt.int16
OP = mybir.AluOpType
AF = mybir.ActivationFunctionType
AX = mybir.AxisListType
DR = mybir.MatmulPerfMode.DoubleRow

# int16 Schraudolph: code = round(A16*psum + B16); code bits ARE the bf16
# representation of ~exp(ESC*psum - SHIFT).  The -7.3616 recenters the
# 2^frac-vs-exp sawtooth so the value-weighted mean ratio is 1 (calibrated
# numerically; residual is ~1.8% noise/elem that averages out per row).
_LOG2E = 1.4426950408889634
A16 = float(np.float32(ESC * _LOG2E * 128.0))
B16 = float(np.float32(128.0 * (127.0 - SHIFT * _LOG2E) - 7.3616))

# drain-mode map per (phase, m): 'S' = wide scalar ACT-exp+accum,
# 'C' = DVE wide int16-Schraudolph TS + DVE fused accum pass,
# 'D' = DVE wide TS + scalar ACT-identity accum over the codes.
# Measured: S = 2.33+0.34us scalar; C = 1.78us + 2.09+0.10us DVE;
# D = 1.78us DVE + 2.00+0.34us scalar.  Tuned so both engines stay
# under the PE fill rate of ~2.07us per m-tile.
MODE = [
    ['C', 'S', 'S', 'C', 'S', 'S', 'C', 'S',
     'S', 'C', 'S', 'S', 'C', 'S', 'S', 'S'],
    ['S', 'C', 'S', 'S', 'C', 'S', 'S', 'C',
     'S', 'S', 'C', 'S', 'S', 'C', 'S', 'S'],
]
PREFIX_M = 2      # m-tiles of phase 0 walked q-major (DMA latency hiding)
N_WARM = 8        # HAM warm-up matmuls (N=512 DoubleRow each, ~0.5us)
TS2_FUSED = True  # use tensor_scalar+accum_out for the chain row-sum


@with_exitstack
def _arc_kernel(ctx: ExitStack, tc: tile.TileContext,
                xt_d: bass.AP, wt_d: bass.AP, s_d: bass.AP):
    nc = tc.nc

    sb = ctx.enter_context(tc.tile_pool(name="sb", bufs=1))
    scr_pool = ctx.enter_context(tc.tile_pool(name="scr", bufs=2))
    ps = ctx.enter_context(tc.tile_pool(name="ps", bufs=2, space="PSUM"))

    xT = sb.tile([P, MT, NJ, 2, P], F8)        # [p, m, j, i, c]
    wT = sb.tile([P, NPIECE, NJ, 2, 512], F8)  # [p, piece, j, i, v]
    Spart = sb.tile([P, NQ, MT], F32)          # per-(ph, m) row sums
    warm = sb.tile([P, 2, 512], F8)            # zeros for HAM warm-up MMs
    zt = sb.tile([P, 1], F32)
    nbias = sb.tile([P, 1], F32)               # -SHIFT bias for the exp

    nc.gpsimd.memset(warm, 0.0)
    nc.vector.memset(Spart, 0.0)
    nc.vector.memset(zt, 0.0)
    nc.vector.memset(nbias, -SHIFT)

    # ---- DMA prefix -------------------------------------------------
    # Two hardware-DGE queues in parallel: sync carries most weight
    # pieces, scalar carries the first x slab + piece 2, so the first
    # m-tiles' operands land with minimum serialization.
    nc.sync.dma_start(out=wT[:, 0, 0], in_=wt_d[:, 0, 0])    # p0 j0
    nc.sync.dma_start(out=xT[:, 0:2], in_=xt_d[:, 0:2])      # x m0-1
    nc.sync.dma_start(out=wT[:, 2], in_=wt_d[:, 2])          # p2
    nc.sync.dma_start(out=xT[:, 2:6], in_=xt_d[:, 2:6])      # x m2-5
    nc.sync.dma_start(out=wT[:, 4], in_=wt_d[:, 4])          # p4
    nc.sync.dma_start(out=wT[:, 6], in_=wt_d[:, 6])          # p6
    nc.sync.dma_start(out=xT[:, 6:11], in_=xt_d[:, 6:11])    # x m6-10
    nc.sync.dma_start(out=xT[:, 11:16], in_=xt_d[:, 11:16])  # x m11-15

    nc.scalar.dma_start(out=wT[:, 0, 1], in_=wt_d[:, 0, 1])  # p0 j1
    nc.scalar.dma_start(out=wT[:, 1], in_=wt_d[:, 1])        # p1
    nc.scalar.dma_start(out=wT[:, 3], in_=wt_d[:, 3])        # p3

    # Force the Exp table load now (~2.9us) so it's resident before the
    # first scalar drain at ~13us.
    e0 = sb.tile([P, 1], F32)
    nc.scalar.activation(out=e0, in_=zt, func=AF.Exp, bias=nbias)

    nc.scalar.dma_start(out=wT[:, 5], in_=wt_d[:, 5])        # p5
    nc.scalar.dma_start(out=wT[:, 7], in_=wt_d[:, 7])        # p7

    # HAM warm-up: real (zero-data) DoubleRow matmuls engage the PE
    # activity monitor while the first weight pieces land.
    wpm = ps.tile([P, 2048], F32, tag="mm", name="warm")
    for _ in range(N_WARM):
        nc.tensor.matmul(wpm[:, 0:512], warm[:, :, 0:128], warm,
                         start=True, stop=True, perf_mode=DR)

    # ---- m-tile pipeline --------------------------------------------
    def fill_span(pm, ph, m, s_list):
        """DoubleRow matmuls for 512-col pieces s_list; j outer so 2-4
        consecutive matmuls share the same stationary x tile."""
        for j in range(NJ):
            for s in s_list:
                pc = 4 * ph + s
                w = PW[pc]
                nc.tensor.matmul(
                    pm[:, 512 * s:512 * s + w],
                    xT[:, m, j],
                    wT[:, pc, j, :, 0:w],
                    start=(j == 0), stop=(j == NJ - 1),
                    perf_mode=DR)

    # dummy SBUF sink for the scalar exp output (we only need accum_out;
    # writing PSUM in-place costs ~300ns of same-bank RW conflict).
    esink = sb.tile([P, 2048], BF16)

    # Chain second passes (code row-sums) are DEFERRED: the reduce of
    # chain m is emitted only after the NEXT chain's psum-freeing TS
    # passes, so it never delays them on the DVE's strict-FIFO queue.
    pending = []

    def finish_chain():
        if not pending:
            return
        ph, m, scr, wtot = pending.pop()
        codes = scr[:, 0:wtot].bitcast(BF16)
        if MODE[ph][m] == 'D':
            nc.scalar.activation(
                out=codes, in_=codes, func=AF.Identity, bias=zt,
                accum_out=Spart[:, 2 * ph, m:m + 1])
        elif TS2_FUSED:
            nc.vector.tensor_scalar(
                codes, codes, 1.0, 0.0, OP.mult, OP.add,
                accum_out=Spart[:, 2 * ph, m:m + 1])
        else:
            nc.vector.tensor_reduce(
                out=Spart[:, 2 * ph, m:m + 1], in_=codes,
                axis=AX.X, op=OP.add)

    def drain_m(ph, m, pm, scr, wtot):
        """Row-sum the filled [128, wtot] psum tile of (ph, m)."""
        mode = MODE[ph][m]
        if mode == 'S':
            nc.scalar.activation(
                out=esink[:, :wtot], in_=pm[:, :wtot], func=AF.Exp,
                bias=nbias, scale=ESC,
                accum_out=Spart[:, 2 * ph, m:m + 1])
            return
        nc.vector.tensor_scalar(scr[:, 0:1024], pm[:, 0:1024],
                                A16, B16, OP.mult, OP.add)
        nc.vector.tensor_scalar(scr[:, 1024:wtot], pm[:, 1024:wtot],
                                A16, B16, OP.mult, OP.add)
        finish_chain()
        pending.append((ph, m, scr, wtot))

    s_view = s_d.rearrange("(p q m) -> p q m", p=P, q=NQ)

    for ph in range(2):
        wtot = 2048 if ph == 0 else 1952
        if ph == 0:
            # prefix: m0/m1 interleaved q-major to hide the weight DMA;
            # per-half (narrow) drains so the psum bufs free in time.
            pms = [ps.tile([P, 2048], F32, tag="mm", name=f"pm{m}")
                   for m in range(PREFIX_M)]
            scrs = [scr_pool.tile([P, 2048], I16, name=f"scr{m}")
                    if MODE[0][m] != 'S' else None for m in range(PREFIX_M)]
            for qq in range(2):
                for m in range(PREFIX_M):
                    fill_span(pms[m], 0, m, (2 * qq, 2 * qq + 1))
                    half = slice(1024 * qq, 1024 * qq + 1024)
                    if MODE[0][m] == 'S':
                        nc.scalar.activation(
                            out=esink[:, 0:1024], in_=pms[m][:, half],
                            func=AF.Exp, bias=nbias, scale=ESC,
                            accum_out=Spart[:, qq, m:m + 1])
                    else:
                        nc.vector.tensor_scalar(
                            scrs[m][:, half], pms[m][:, half],
                            A16, B16, OP.mult, OP.add)
            for m in range(PREFIX_M):
                if MODE[0][m] != 'S':
                    pending.append((0, m, scrs[m], wtot))
            ms = range(PREFIX_M, MT)
        else:
            ms = range(MT)
        for m in ms:
            pm = ps.tile([P, 2048], F32, tag="mm", name="pm")
            scr = (scr_pool.tile([P, 2048], I16, name="scr")
                   if MODE[ph][m] != 'S' else None)
            if ph == 1 and m == MT - 1:
                # last m-tile: per-half drains so the post-matmul tail is
                # a single narrow ACT
                fill_span(pm, 1, m, (0, 1))
                nc.scalar.activation(
                    out=esink[:, 0:1024], in_=pm[:, 0:1024],
                    func=AF.Exp, bias=nbias, scale=ESC,
                    accum_out=Spart[:, 2, m:m + 1])
                fill_span(pm, 1, m, (2, 3))
                nc.scalar.activation(
                    out=esink[:, 0:wtot - 1024], in_=pm[:, 1024:wtot],
                    func=AF.Exp, bias=nbias, scale=ESC,
                    accum_out=Spart[:, 3, m:m + 1])
                continue
            fill_span(pm, ph, m, (0, 1, 2, 3))
            drain_m(ph, m, pm, scr, wtot)
        finish_chain()
        # ship this phase's sums while the next phase runs
        nc.sync.dma_start(out=s_view[:, 2 * ph:2 * ph + 2],
                          in_=Spart[:, 2 * ph:2 * ph + 2])


def build_bass():
    nc = bacc.Bacc("TRN2", target_bir_lowering=False, debug=False,
                   enable_asserts=False, num_devices=NCORES)
    xt_d = nc.dram_tensor("xt_in", [P, MT, NJ, 2, P], F8,
                          kind="ExternalInput").ap()
    wt_d = nc.dram_tensor("wt_in", [P, NPIECE, NJ, 2, 512], F8,
                          kind="ExternalInput").ap()
    s_d = nc.dram_tensor("s_out", [NQ * B], F32, kind="ExternalOutput").ap()
    with tile.TileContext(nc) as tc:
        _arc_kernel(tc, xt_d, wt_d, s_d)
    nc.compile()
    return nc


_NC = None


def _get_nc():
    global _NC
    if _NC is None:
        _NC = build_bass()
    return _NC


def make_in_maps(xn: np.ndarray, W: np.ndarray):
    import ml_dtypes
    F8NP = ml_dtypes.float8_e4m3

    xq = (xn * WS).astype(F8NP)                      # [B, D]
    # xt[p, m, j, i, c] = xq[m*128 + c, j*256 + i*128 + p]
    xt = np.ascontiguousarray(
        xq.reshape(MT, P, NJ, 2, P).transpose(4, 0, 2, 3, 1))

    wnorm = np.linalg.norm(W, axis=1, keepdims=True)
    Wn = W / np.maximum(wnorm, EPS)
    in_maps = []
    for c in range(NCORES):
        wq = np.zeros((VSP, D), dtype=F8NP)
        wq[:VS] = (Wn[c * VS:(c + 1) * VS] * WS).astype(F8NP)
        # wt[p, piece, j, i, v] = wq[piece*512 + v, j*256 + i*128 + p]
        wt = np.ascontiguousarray(
            wq.reshape(NPIECE, 512, NJ, 2, P).transpose(4, 0, 2, 3, 1))
        in_maps.append({"xt_in": xt, "wt_in": wt})
    return in_maps, Wn


def kernel(x, W, labels, **run_kwargs):
    x = np.ascontiguousarray(np.asarray(x), dtype=np.float32)
    W = np.ascontiguousarray(np.asarray(W), dtype=np.float32)
    lab = np.asarray(labels).astype(np.int64)
    assert x.shape == (B, D) and W.shape == (V, D) and lab.shape == (B,), \
        (x.shape, W.shape, lab.shape)

    xn = x / np.maximum(np.linalg.norm(x, axis=1, keepdims=True), EPS)

    nc = _get_nc()
    in_maps, Wn = make_in_maps(xn, W)
    res = bass_utils.run_bass_kernel_spmd(
        nc, in_maps, core_ids=list(range(NCORES)), **run_kwargs)

    S = np.zeros(B, dtype=np.float64)
    for r in res.results:
        sp = r["s_out"].reshape(P, NQ, MT).sum(axis=1)  # add the quarters
        S += sp.T.reshape(-1).astype(np.float64)

    # Host label-column correction (O(B*D), fp64 epilogue).
    cos_y = np.einsum("bd,bd->b", xn.astype(np.float64),
                      Wn[lab].astype(np.float64))
    sin_y = np.sqrt(np.clip(1.0 - cos_y * cos_y, 0.0, 1.0))
    phi_y = cos_y * math.cos(M_MARGIN) - sin_y * math.sin(M_MARGIN)
    S += np.exp(S_SCALE * phi_y - SHIFT) - np.exp(S_SCALE * cos_y - SHIFT)
    loss = np.mean(SHIFT + np.log(S) - S_SCALE * phi_y)

    kernel.last_results = res
    return np.asarray(loss, dtype=np.float32)


# revision 15
# speedup vs baseline: 1.0931x; 1.0931x over previous
"""ArcFace margin loss (ArcMarginLoss) on 8 Trainium2 NeuronCores.

Classification-parallel sharding: V=32000 classes split across 8 cores
(4000 each, padded to 4096).  The device kernel is a pure fp8 GEMM +
exp-rowsum pipeline; everything O(B*D) or O(V*D)-elementwise lives on the
host.

Host prep (numpy):
  - x-hat = x/|x|, w-hat = W/|W| rows (fp32), scaled by 16 and cast to
    fp8 e4m3.  PSUM then holds 256*cos, and the exp stage applies scale
    s/256 and bias -s, computing exp(s*cos - 30) directly: since cos <= 1
    no global max pass or cross-core collective is needed.
  - Both operands are packed K-major for the PE's fp8 DoubleRow mode
    (2x MAC throughput).  Weights are packed piece-major
    ([P, piece, j, i, 512]) so each 512-class piece is one
    contiguous-per-partition DMA.
  - The label-column path (cos_y, phi, per-row exp corrections) is
    O(B*D), computed on the host in fp64.

Device per core (one SPMD NEFF), v3 pipeline:
  - Two phases of 2 class-quarters each.  Per (phase, m-tile): a single
    [128, 2048] fp32 PSUM tile (4 banks; 2-buf ring = whole PSUM) filled
    by 8 DoubleRow matmuls (4 x 512-col pieces x 2 k-planes).
  - Drains are split per m-tile between the two PSUM-capable engines
    (GpSimd has no PSUM port on TRN2):
      * 'S': ONE wide scalar ACT Exp over all 2048 cols with fused accum
        row-sum (amortizes the 352-cycle ACT ramp + 344ns accumulator
        read); in-place, and the 2-buf ring leaves 2072ns of drain
        latency slack so the 2344ns ACT fits without stalling the PE.
      * 'C': DVE tensor_scalar converts each 1024-col half of the s*cos
        logits straight to int16 bf16 BIT CODES (Schraudolph exp in the
        bf16 domain) into SBUF scratch -- the PSUM halves free after
        ~1.6us each -- then one fused tensor_scalar(mult 1, add 0) with
        accum_out over the codes read as bf16 yields the row-sum without
        a separate reduce pass.
    The (ph, m) -> engine map is tuned so both engines stay below the PE
    fill rate (~2072ns per m).
  - The first two m-tiles of phase 0 are walked q-major to hide the
    initial weight-DMA latency; DMA issues are split across the sync and
    scalar hardware-DGE queues so the critical pieces transfer in
    parallel.
  - HAM warm-up: real DoubleRow matmuls on a zeroed fp8 tile ramp the PE
    clock gate (transposes don't count as PE-busy for HAM).
  - Output: per-(phase, m) row sums Spart [128, 4, 16] fp32, DMA'd per
    phase.

Host epilogue: S = sum_c sum_q S_cq, scatter-add the label corrections,
loss = mean(30 + log(S) - s*phi_label).
"""

import math
import numpy as np
from contextlib import ExitStack

import concourse.bass as bass
import concourse.tile as tile
from concourse import bacc, mybir
from concourse import bass_utils
from concourse._compat import with_exitstack

P = 128
B = 2048          # batch rows
D = 512           # feature dim
V = 32000         # classes
NCORES = 8
VS = V // NCORES  # 4000 classes per core
VSP = 4096        # padded shard size
MT = B // P       # 16 batch row tiles
NJ = 2            # DoubleRow passes over D (each contracts 256)
NQ = 4            # class quarters per core
NPIECE = 8        # 512-class weight pieces per core
PW = (512, 512, 512, 512, 512, 512, 512, 416)  # real cols per piece

S_SCALE = 30.0
M_MARGIN = 0.5
SHIFT = 30.0      # exp(logit - SHIFT): logits <= 30 so always <= 0
WS = 16.0         # fp8 encode scale for x-hat and w-hat
EPS = 1e-12
ESC = S_SCALE / (WS * WS)   # psum -> logit scale (psum = 256*cos)

F32 = mybir.dt.float32
BF16 = mybir.dt.bfloat16
F8 = mybir.dt.float8e4
I16 = mybir.dt.int16
OP = mybir.AluOpType
AF = mybir.ActivationFunctionType
AX = mybir.AxisListType
DR = mybir.MatmulPerfMode.DoubleRow

# int16 Schraudolph: code = round(A16*psum + B16); code bits ARE the bf16
# representation of ~exp(ESC*psum - SHIFT).  The -7.3616 recenters the
# 2^frac-vs-exp sawtooth so the value-weighted mean ratio is 1 (calibrated
# numerically; residual is ~1.8% noise/elem that averages out per row).
_LOG2E = 1.4426950408889634
A16 = float(np.float32(ESC * _LOG2E * 128.0))
B16 = float(np.float32(128.0 * (127.0 - SHIFT * _LOG2E) - 7.3616))

# drain-mode map per (phase, m): 'S' = wide scalar ACT-exp+accum,
# 'C' = DVE wide int16-Schraudolph TS + DVE fused accum pass,
# 'D' = DVE wide TS + scalar ACT-identity accum over the codes.
# Measured: S = 2.33+0.34us scalar; C = 1.78us + 2.09+0.10us DVE;
# D = 1.78us DVE + 2.00+0.34us scalar.  Tuned so both engines stay
# under the PE fill rate of ~2.07us per m-tile.
MODE = [
    ['C', 'S', 'S', 'C', 'S', 'C', 'S', 'S',
     'C', 'S', 'S', 'C', 'S', 'S', 'C', 'S'],
    ['S', 'C', 'S', 'S', 'C', 'S', 'S', 'C',
     'S', 'S', 'C', 'S', 'C', 'S', 'C', 'S'],
]
PREFIX_M = 2      # m-tiles of phase 0 walked q-major (DMA latency hiding)
N_WARM = 8        # HAM warm-up matmuls (N=512 DoubleRow each, ~0.5us)
TS2_FUSED = True  # use tensor_scalar+accum_out for the chain row-sum


@with_exitstack
def _arc_kernel(ctx: ExitStack, tc: tile.TileContext,
                xt_d: bass.AP, wt_d: bass.AP, s_d: bass.AP):
    nc = tc.nc

    sb = ctx.enter_context(tc.tile_pool(name="sb", bufs=1))
    scr_pool = ctx.enter_context(tc.tile_pool(name="scr", bufs=2))
    ps = ctx.enter_context(tc.tile_pool(name="ps", bufs=4, space="PSUM"))

    xT = sb.tile([P, MT, NJ, 2, P], F8)        # [p, m, j, i, c]
    wT = sb.tile([P, NPIECE, NJ, 2, 512], F8)  # [p, piece, j, i, v]
    Spart = sb.tile([P, NQ, MT], F32)          # per-(ph, m) row sums
    warm = sb.tile([P, 2, 512], F8)            # zeros for HAM warm-up MMs
    zt = sb.tile([P, 1], F32)
    nbias = sb.tile([P, 1], F32)               # -SHIFT bias for the exp

    nc.gpsimd.memset(warm, 0.0)
    nc.vector.memset(Spart, 0.0)
    nc.vector.memset(zt, 0.0)
    nc.vector.memset(nbias, -SHIFT)

    # ---- DMA prefix -------------------------------------------------
    # Two hardware-DGE queues in parallel: sync carries most weight
    # pieces, scalar carries the first x slab + piece 2, so the first
    # m-tiles' operands land with minimum serialization.
    nc.sync.dma_start(out=wT[:, 0, 0], in_=wt_d[:, 0, 0])    # p0 j0
    nc.sync.dma_start(out=xT[:, 0:2], in_=xt_d[:, 0:2])      # x m0-1
    nc.sync.dma_start(out=wT[:, 2], in_=wt_d[:, 2])          # p2
    nc.sync.dma_start(out=xT[:, 2:6], in_=xt_d[:, 2:6])      # x m2-5
    nc.sync.dma_start(out=wT[:, 4], in_=wt_d[:, 4])          # p4
    nc.sync.dma_start(out=wT[:, 6], in_=wt_d[:, 6])          # p6
    nc.sync.dma_start(out=xT[:, 6:11], in_=xt_d[:, 6:11])    # x m6-10
    nc.sync.dma_start(out=xT[:, 11:16], in_=xt_d[:, 11:16])  # x m11-15

    nc.scalar.dma_start(out=wT[:, 0, 1], in_=wt_d[:, 0, 1])  # p0 j1
    nc.scalar.dma_start(out=wT[:, 1], in_=wt_d[:, 1])        # p1
    nc.scalar.dma_start(out=wT[:, 3], in_=wt_d[:, 3])        # p3

    # Force the Exp table load now (~2.9us) so it's resident before the
    # first scalar drain at ~13us.
    e0 = sb.tile([P, 1], F32)
    nc.scalar.activation(out=e0, in_=zt, func=AF.Exp, bias=nbias)

    nc.scalar.dma_start(out=wT[:, 5], in_=wt_d[:, 5])        # p5
    nc.scalar.dma_start(out=wT[:, 7], in_=wt_d[:, 7])        # p7

    # HAM warm-up: real (zero-data) DoubleRow matmuls engage the PE
    # activity monitor while the first weight pieces land.
    wpm = ps.tile([P, 1024], F32, tag="mm", name="warm")
    for _ in range(N_WARM):
        nc.tensor.matmul(wpm[:, 0:512], warm[:, :, 0:128], warm,
                         start=True, stop=True, perf_mode=DR)

    # ---- chunk pipeline ---------------------------------------------
    # Chunk = (m, half) = [128, <=1024] psum tile (2 banks; 4-buf ring).
    # j inner: the ~259ns/MM pace matches the aggregate drain rate, and
    # the 4-ring gives each drain ~3 chunk-periods of latency slack.
    def fill_chunk(pm, ph, m, h):
        for s in (2 * h, 2 * h + 1):
            pc = 4 * ph + s
            w = PW[pc]
            lo = 512 * (s - 2 * h)
            for j in range(NJ):
                nc.tensor.matmul(
                    pm[:, lo:lo + w],
                    xT[:, m, j],
                    wT[:, pc, j, :, 0:w],
                    start=(j == 0), stop=(j == NJ - 1),
                    perf_mode=DR)

    # Chain second passes (code row-sums) are DEFERRED: the reduce of
    # chain m is emitted only after the NEXT chain's psum-freeing TS
    # passes, so it never delays them on the DVE's strict-FIFO queue.
    pending = []

    def finish_chain():
        if not pending:
            return
        ph, m, scr, wtot = pending.pop()
        codes = scr[:, 0:wtot].bitcast(BF16)
        if MODE[ph][m] == 'D':
            nc.scalar.activation(
                out=codes, in_=codes, func=AF.Identity, bias=zt,
                accum_out=Spart[:, 2 * ph, m:m + 1])
        elif TS2_FUSED:
            nc.vector.tensor_scalar(
                codes, codes, 1.0, 0.0, OP.mult, OP.add,
                accum_out=Spart[:, 2 * ph, m:m + 1])
        else:
            nc.vector.tensor_reduce(
                out=Spart[:, 2 * ph, m:m + 1], in_=codes,
                axis=AX.X, op=OP.add)

    def drain_chunk(ph, m, h, pm, scr, w):
        """Drain one [128, w] psum chunk of (m, half h)."""
        if MODE[ph][m] == 'S':
            nc.scalar.activation(
                out=pm[:, :w], in_=pm[:, :w], func=AF.Exp,
                bias=nbias, scale=ESC,
                accum_out=Spart[:, 2 * ph + h, m:m + 1])
        else:
            nc.vector.tensor_scalar(scr[:, 1024 * h:1024 * h + w],
                                    pm[:, :w], A16, B16, OP.mult, OP.add)

    s_view = s_d.rearrange("(p q m) -> p q m", p=P, q=NQ)

    def chunk_w(ph, h):
        return PW[4 * ph + 2 * h] + PW[4 * ph + 2 * h + 1]

    for ph in range(2):
        wtot = chunk_w(ph, 0) + chunk_w(ph, 1)
        if ph == 0:
            # prefix: m0/m1 walked half-major (q-major) to hide the
            # initial weight-piece DMA latency.
            scrs = [scr_pool.tile([P, 2048], I16, name=f"scr{m}")
                    if MODE[0][m] != 'S' else None for m in range(PREFIX_M)]
            for h in range(2):
                for m in range(PREFIX_M):
                    pm = ps.tile([P, 1024], F32, tag="mm", name="pm")
                    fill_chunk(pm, 0, m, h)
                    drain_chunk(0, m, h, pm, scrs[m], chunk_w(0, h))
            for m in range(PREFIX_M):
                if MODE[0][m] != 'S':
                    pending.append((0, m, scrs[m], wtot))
            ms = range(PREFIX_M, MT)
        else:
            ms = range(MT)
        for m in ms:
            scr = (scr_pool.tile([P, 2048], I16, name="scr")
                   if MODE[ph][m] != 'S' else None)
            for h in range(2):
                pm = ps.tile([P, 1024], F32, tag="mm", name="pm")
                fill_chunk(pm, ph, m, h)
                drain_chunk(ph, m, h, pm, scr, chunk_w(ph, h))
            if scr is not None:
                finish_chain()
                pending.append((ph, m, scr, wtot))
        finish_chain()
        # ship this phase's sums while the next phase runs
        nc.sync.dma_start(out=s_view[:, 2 * ph:2 * ph + 2],
                          in_=Spart[:, 2 * ph:2 * ph + 2])


def build_bass():
    nc = bacc.Bacc("TRN2", target_bir_lowering=False, debug=False,
                   enable_asserts=False, num_devices=NCORES)
    xt_d = nc.dram_tensor("xt_in", [P, MT, NJ, 2, P], F8,
                          kind="ExternalInput").ap()
    wt_d = nc.dram_tensor("wt_in", [P, NPIECE, NJ, 2, 512], F8,
                          kind="ExternalInput").ap()
    s_d = nc.dram_tensor("s_out", [NQ * B], F32, kind="ExternalOutput").ap()
    with tile.TileContext(nc) as tc:
        _arc_kernel(tc, xt_d, wt_d, s_d)
    nc.compile()
    return nc


_NC = None


def _get_nc():
    global _NC
    if _NC is None:
        _NC = build_bass()
    return _NC


def make_in_maps(xn: np.ndarray, W: np.ndarray):
    import ml_dtypes
    F8NP = ml_dtypes.float8_e4m3

    xq = (xn * WS).astype(F8NP)                      # [B, D]
    # xt[p, m, j, i, c] = xq[m*128 + c, j*256 + i*128 + p]
    xt = np.ascontiguousarray(
        xq.reshape(MT, P, NJ, 2, P).transpose(4, 0, 2, 3, 1))

    wnorm = np.linalg.norm(W, axis=1, keepdims=True)
    Wn = W / np.maximum(wnorm, EPS)
    in_maps = []
    for c in range(NCORES):
        wq = np.zeros((VSP, D), dtype=F8NP)
        wq[:VS] = (Wn[c * VS:(c + 1) * VS] * WS).astype(F8NP)
        # wt[p, piece, j, i, v] = wq[piece*512 + v, j*256 + i*128 + p]
        wt = np.ascontiguousarray(
            wq.reshape(NPIECE, 512, NJ, 2, P).transpose(4, 0, 2, 3, 1))
        in_maps.append({"xt_in": xt, "wt_in": wt})
    return in_maps, Wn


def kernel(x, W, labels, **run_kwargs):
    x = np.ascontiguousarray(np.asarray(x), dtype=np.float32)
    W = np.ascontiguousarray(np.asarray(W), dtype=np.float32)
    lab = np.asarray(labels).astype(np.int64)
    assert x.shape == (B, D) and W.shape == (V, D) and lab.shape == (B,), \
        (x.shape, W.shape, lab.shape)

    xn = x / np.maximum(np.linalg.norm(x, axis=1, keepdims=True), EPS)

    nc = _get_nc()
    in_maps, Wn = make_in_maps(xn, W)
    res = bass_utils.run_bass_kernel_spmd(
        nc, in_maps, core_ids=list(range(NCORES)), **run_kwargs)

    S = np.zeros(B, dtype=np.float64)
    for r in res.results:
        sp = r["s_out"].reshape(P, NQ, MT).sum(axis=1)  # add the quarters
        S += sp.T.reshape(-1).astype(np.float64)

    # Host label-column correction (O(B*D), fp64 epilogue).
    cos_y = np.einsum("bd,bd->b", xn.astype(np.float64),
                      Wn[lab].astype(np.float64))
    sin_y = np.sqrt(np.clip(1.0 - cos_y * cos_y, 0.0, 1.0))
    phi_y = cos_y * math.cos(M_MARGIN) - sin_y * math.sin(M_MARGIN)
    S += np.exp(S_SCALE * phi_y - SHIFT) - np.exp(S_SCALE * cos_y - SHIFT)
    loss = np.mean(SHIFT + np.log(S) - S_SCALE * phi_y)

    kernel.last_results = res
    return np.asarray(loss, dtype=np.float32)


# revision 19
# speedup vs baseline: 1.1230x; 1.0274x over previous
"""ArcFace margin loss (ArcMarginLoss) on 8 Trainium2 NeuronCores.

Classification-parallel sharding: V=32000 classes split across 8 cores
(4000 each, padded to 4096).  The device kernel is a pure fp8 GEMM +
exp-rowsum pipeline; everything O(B*D) or O(V*D)-elementwise lives on the
host.

Host prep (numpy):
  - x-hat = x/|x|, w-hat = W/|W| rows (fp32), scaled by 16 and cast to
    fp8 e4m3.  PSUM then holds 256*cos, and the exp stage applies scale
    s/256 and bias -s, computing exp(s*cos - 30) directly: since cos <= 1
    no global max pass or cross-core collective is needed.
  - Both operands are packed K-major for the PE's fp8 DoubleRow mode
    (2x MAC throughput).  Weights are packed piece-major
    ([P, piece, j, i, 512]) so each 512-class piece is one
    contiguous-per-partition DMA.
  - The label-column path (cos_y, phi, per-row exp corrections) is
    O(B*D), computed on the host in fp64.

Device per core (one SPMD NEFF), v3 pipeline:
  - Two phases of 2 class-quarters each.  Per (phase, m-tile): a single
    [128, 2048] fp32 PSUM tile (4 banks; 2-buf ring = whole PSUM) filled
    by 8 DoubleRow matmuls (4 x 512-col pieces x 2 k-planes).
  - Drains are split per m-tile between the two PSUM-capable engines
    (GpSimd has no PSUM port on TRN2):
      * 'S': ONE wide scalar ACT Exp over all 2048 cols with fused accum
        row-sum (amortizes the 352-cycle ACT ramp + 344ns accumulator
        read); in-place, and the 2-buf ring leaves 2072ns of drain
        latency slack so the 2344ns ACT fits without stalling the PE.
      * 'C': DVE tensor_scalar converts each 1024-col half of the s*cos
        logits straight to int16 bf16 BIT CODES (Schraudolph exp in the
        bf16 domain) into SBUF scratch -- the PSUM halves free after
        ~1.6us each -- then one fused tensor_scalar(mult 1, add 0) with
        accum_out over the codes read as bf16 yields the row-sum without
        a separate reduce pass.
    The (ph, m) -> engine map is tuned so both engines stay below the PE
    fill rate (~2072ns per m).
  - The first two m-tiles of phase 0 are walked q-major to hide the
    initial weight-DMA latency; DMA issues are split across the sync and
    scalar hardware-DGE queues so the critical pieces transfer in
    parallel.
  - HAM warm-up: real DoubleRow matmuls on a zeroed fp8 tile ramp the PE
    clock gate (transposes don't count as PE-busy for HAM).
  - Output: per-(phase, m) row sums Spart [128, 4, 16] fp32, DMA'd per
    phase.

Host epilogue: S = sum_c sum_q S_cq, scatter-add the label corrections,
loss = mean(30 + log(S) - s*phi_label).
"""

import math
import numpy as np
from contextlib import ExitStack

import concourse.bass as bass
import concourse.tile as tile
from concourse import bacc, mybir
from concourse import bass_utils
from concourse._compat import with_exitstack

P = 128
B = 2048          # batch rows
D = 512           # feature dim
V = 32000         # classes
NCORES = 8
VS = V // NCORES  # 4000 classes per core
VSP = 4096        # padded shard size
MT = B // P       # 16 batch row tiles
NJ = 2            # DoubleRow passes over D (each contracts 256)
NQ = 4            # class quarters per core
NPIECE = 8        # 512-class weight pieces per core
PW = (512, 512, 512, 512, 512, 512, 512, 416)  # real cols per piece

S_SCALE = 30.0
M_MARGIN = 0.5
SHIFT = 30.0      # exp(logit - SHIFT): logits <= 30 so always <= 0
WS = 16.0         # fp8 encode scale for x-hat and w-hat
EPS = 1e-12
ESC = S_SCALE / (WS * WS)   # psum -> logit scale (psum = 256*cos)

F32 = mybir.dt.float32
BF16 = mybir.dt.bfloat16
F8 = mybir.dt.float8e4
I16 = mybir.dt.int16
OP = mybir.AluOpType
AF = mybir.ActivationFunctionType
AX = mybir.AxisListType
DR = mybir.MatmulPerfMode.DoubleRow

# int16 Schraudolph: code = round(A16*psum + B16); code bits ARE the bf16
# representation of ~exp(ESC*psum - SHIFT).  The -7.3616 recenters the
# 2^frac-vs-exp sawtooth so the value-weighted mean ratio is 1 (calibrated
# numerically; residual is ~1.8% noise/elem that averages out per row).
_LOG2E = 1.4426950408889634
A16 = float(np.float32(ESC * _LOG2E * 128.0))
B16 = float(np.float32(128.0 * (127.0 - SHIFT * _LOG2E) - 7.3616))

# drain-mode map per (phase, m): 'S' = wide scalar ACT-exp+accum,
# 'C' = DVE wide int16-Schraudolph TS + DVE fused accum pass,
# 'D' = DVE wide TS + scalar ACT-identity accum over the codes.
# Measured: S = 2.33+0.34us scalar; C = 1.78us + 2.09+0.10us DVE;
# D = 1.78us DVE + 2.00+0.34us scalar.  Tuned so both engines stay
# under the PE fill rate of ~2.07us per m-tile.
MODE = [
    ['C', 'S', 'C', 'S', 'S', 'C', 'S', 'C',
     'S', 'S', 'C', 'S', 'C', 'S', 'C', 'S'],
    ['S', 'C', 'S', 'C', 'S', 'S', 'C', 'S',
     'C', 'S', 'C', 'S', 'C', 'S', 'C', 'S'],
]
PREFIX_M = 2      # m-tiles of phase 0 walked q-major (DMA latency hiding)
N_WARM = 8        # HAM warm-up matmuls (N=512 DoubleRow each, ~0.5us)
TS2_FUSED = True  # use tensor_scalar+accum_out for the chain row-sum


@with_exitstack
def _arc_kernel(ctx: ExitStack, tc: tile.TileContext,
                xt_d: bass.AP, wt_d: bass.AP, s_d: bass.AP):
    nc = tc.nc

    sb = ctx.enter_context(tc.tile_pool(name="sb", bufs=1))
    scr_pool = ctx.enter_context(tc.tile_pool(name="scr", bufs=2))
    scr2_pool = ctx.enter_context(tc.tile_pool(name="scr2", bufs=2))
    # Flat PSUM: one [128, 4096] fp32 tensor = all 8 banks, managed as a
    # manual 4-slot ring of 1024-col chunks.  An m-tile's two chunks land
    # on adjacent slots (offsets 0+1024 or 2048+3072), so a single wide
    # scalar ACT can drain both and still release them with ~3
    # chunk-periods of latency slack (range-level dependency tracking).
    pmall = nc.alloc_psum_tensor("pmall", [P, 4096], F32).ap()

    xT = sb.tile([P, MT, NJ, 2, P], F8)        # [p, m, j, i, c]
    wT = sb.tile([P, NPIECE, NJ, 2, 512], F8)  # [p, piece, j, i, v]
    Spart = sb.tile([P, NQ, MT], F32)          # per-(ph, m) row sums
    warm = sb.tile([P, 2, 512], F8)            # zeros for HAM warm-up MMs
    zt = sb.tile([P, 1], F32)
    nbias = sb.tile([P, 1], F32)               # -SHIFT bias for the exp

    nc.gpsimd.memset(warm, 0.0)
    nc.vector.memset(Spart, 0.0)
    nc.vector.memset(zt, 0.0)
    nc.vector.memset(nbias, -SHIFT)

    # ---- DMA prefix -------------------------------------------------
    # Two hardware-DGE queues in parallel: sync carries most weight
    # pieces, scalar carries the first x slab + piece 2, so the first
    # m-tiles' operands land with minimum serialization.
    nc.sync.dma_start(out=wT[:, 0, 0], in_=wt_d[:, 0, 0])    # p0 j0
    nc.sync.dma_start(out=xT[:, 0:2], in_=xt_d[:, 0:2])      # x m0-1
    nc.sync.dma_start(out=wT[:, 2], in_=wt_d[:, 2])          # p2
    nc.sync.dma_start(out=xT[:, 2:6], in_=xt_d[:, 2:6])      # x m2-5
    nc.sync.dma_start(out=wT[:, 4], in_=wt_d[:, 4])          # p4
    nc.sync.dma_start(out=wT[:, 6], in_=wt_d[:, 6])          # p6
    nc.sync.dma_start(out=xT[:, 6:11], in_=xt_d[:, 6:11])    # x m6-10
    nc.sync.dma_start(out=xT[:, 11:16], in_=xt_d[:, 11:16])  # x m11-15

    nc.scalar.dma_start(out=wT[:, 0, 1], in_=wt_d[:, 0, 1])  # p0 j1
    nc.scalar.dma_start(out=wT[:, 1], in_=wt_d[:, 1])        # p1
    nc.scalar.dma_start(out=wT[:, 3], in_=wt_d[:, 3])        # p3

    # Force the Exp table load now (~2.9us) so it's resident before the
    # first scalar drain at ~13us.
    e0 = sb.tile([P, 1], F32)
    nc.scalar.activation(out=e0, in_=zt, func=AF.Exp, bias=nbias)

    nc.scalar.dma_start(out=wT[:, 5], in_=wt_d[:, 5])        # p5
    nc.scalar.dma_start(out=wT[:, 7], in_=wt_d[:, 7])        # p7

    # HAM warm-up: real (zero-data) DoubleRow matmuls engage the PE
    # activity monitor while the first weight pieces land.
    for _ in range(N_WARM):
        nc.tensor.matmul(pmall[:, 0:512], warm[:, :, 0:128], warm,
                         start=True, stop=True, perf_mode=DR)

    # ---- chunk pipeline ---------------------------------------------
    # Chunk = (m, half) = [128, <=1024] window of the flat psum ring.
    def fill_chunk(base, ph, m, h):
        for s in (2 * h, 2 * h + 1):
            pc = 4 * ph + s
            w = PW[pc]
            lo = base + 512 * (s - 2 * h)
            for j in range(NJ):
                nc.tensor.matmul(
                    pmall[:, lo:lo + w],
                    xT[:, m, j],
                    wT[:, pc, j, :, 0:w],
                    start=(j == 0), stop=(j == NJ - 1),
                    perf_mode=DR)

    # Chain finishers: the gpsimd TT halves the codes right away (its
    # queue is otherwise idle); the DVE accum pass over the halved codes
    # is DEFERRED until after the NEXT chain's psum-freeing TS passes so
    # it never delays them on the DVE's strict-FIFO queue.
    pending = []

    def finish_chain():
        if not pending:
            return
        ph, m, scr2, wh = pending.pop()
        nc.vector.tensor_scalar(
            scr2[:, 0:wh], scr2[:, 0:wh], 1.0, 0.0, OP.mult, OP.add,
            accum_out=Spart[:, 2 * ph, m:m + 1])

    def chain_tail(ph, m, scr, wtot):
        """gpsimd-halve the bf16 codes of chain (ph, m), defer the sum."""
        wh = wtot // 2
        scr2 = scr2_pool.tile([P, 1024], BF16, name="scr2")
        nc.gpsimd.tensor_tensor(
            out=scr2[:, 0:wh], in0=scr[:, 0:wh].bitcast(BF16),
            in1=scr[:, wh:wtot].bitcast(BF16), op=OP.add)
        finish_chain()
        pending.append((ph, m, scr2, wh))

    def drain_chunk(base, ph, m, h, scr, w):
        """Drain one [128, w] psum chunk of (m, half h) -- narrow ops."""
        if MODE[ph][m] == 'S':
            nc.scalar.activation(
                out=pmall[:, base:base + w], in_=pmall[:, base:base + w],
                func=AF.Exp, bias=nbias, scale=ESC,
                accum_out=Spart[:, 2 * ph + h, m:m + 1])
        else:
            nc.vector.tensor_scalar(scr[:, 1024 * h:1024 * h + w],
                                    pmall[:, base:base + w],
                                    A16, B16, OP.mult, OP.add)

    s_view = s_d.rearrange("(p q m) -> p q m", p=P, q=NQ)

    def chunk_w(ph, h):
        return PW[4 * ph + 2 * h] + PW[4 * ph + 2 * h + 1]

    chunk_idx = 0
    for ph in range(2):
        wtot = chunk_w(ph, 0) + chunk_w(ph, 1)
        if ph == 0:
            # prefix: m0/m1 walked half-major (q-major) to hide the
            # initial weight-piece DMA latency; narrow drains.
            scrs = [scr_pool.tile([P, 2048], I16, name=f"scr{m}")
                    if MODE[0][m] != 'S' else None for m in range(PREFIX_M)]
            for h in range(2):
                for m in range(PREFIX_M):
                    base = (chunk_idx % 4) * 1024
                    chunk_idx += 1
                    fill_chunk(base, 0, m, h)
                    drain_chunk(base, 0, m, h, scrs[m], chunk_w(0, h))
            for m in range(PREFIX_M):
                if MODE[0][m] != 'S':
                    chain_tail(0, m, scrs[m], wtot)
            ms = range(PREFIX_M, MT)
        else:
            ms = range(MT)
        for m in ms:
            base = (chunk_idx % 4) * 1024
            chunk_idx += 2
            if m == MT - 1:
                # flush the pending chain sum so it overlaps the last
                # m-tile instead of trailing the final matmul
                finish_chain()
            scr = (scr_pool.tile([P, 2048], I16, name="scr")
                   if MODE[ph][m] != 'S' else None)
            fill_chunk(base, ph, m, 0)
            drain_chunk(base, ph, m, 0, scr, chunk_w(ph, 0))
            fill_chunk(base + 1024, ph, m, 1)
            drain_chunk(base + 1024, ph, m, 1, scr, chunk_w(ph, 1))
            if scr is not None:
                chain_tail(ph, m, scr, wtot)
        finish_chain()
        # ship this phase's sums while the next phase runs
        nc.sync.dma_start(out=s_view[:, 2 * ph:2 * ph + 2],
                          in_=Spart[:, 2 * ph:2 * ph + 2])


def build_bass():
    nc = bacc.Bacc("TRN2", target_bir_lowering=False, debug=False,
                   enable_asserts=False, num_devices=NCORES)
    xt_d = nc.dram_tensor("xt_in", [P, MT, NJ, 2, P], F8,
                          kind="ExternalInput").ap()
    wt_d = nc.dram_tensor("wt_in", [P, NPIECE, NJ, 2, 512], F8,
                          kind="ExternalInput").ap()
    s_d = nc.dram_tensor("s_out", [NQ * B], F32, kind="ExternalOutput").ap()
    with tile.TileContext(nc) as tc:
        _arc_kernel(tc, xt_d, wt_d, s_d)
    nc.compile()
    return nc


_NC = None


def _get_nc():
    global _NC
    if _NC is None:
        _NC = build_bass()
    return _NC


def make_in_maps(xn: np.ndarray, W: np.ndarray):
    import ml_dtypes
    F8NP = ml_dtypes.float8_e4m3

    xq = (xn * WS).astype(F8NP)                      # [B, D]
    # xt[p, m, j, i, c] = xq[m*128 + c, j*256 + i*128 + p]
    xt = np.ascontiguousarray(
        xq.reshape(MT, P, NJ, 2, P).transpose(4, 0, 2, 3, 1))

    wnorm = np.linalg.norm(W, axis=1, keepdims=True)
    Wn = W / np.maximum(wnorm, EPS)
    in_maps = []
    for c in range(NCORES):
        wq = np.zeros((VSP, D), dtype=F8NP)
        wq[:VS] = (Wn[c * VS:(c + 1) * VS] * WS).astype(F8NP)
        # wt[p, piece, j, i, v] = wq[piece*512 + v, j*256 + i*128 + p]
        wt = np.ascontiguousarray(
            wq.reshape(NPIECE, 512, NJ, 2, P).transpose(4, 0, 2, 3, 1))
        in_maps.append({"xt_in": xt, "wt_in": wt})
    return in_maps, Wn


def kernel(x, W, labels, **run_kwargs):
    x = np.ascontiguousarray(np.asarray(x), dtype=np.float32)
    W = np.ascontiguousarray(np.asarray(W), dtype=np.float32)
    lab = np.asarray(labels).astype(np.int64)
    assert x.shape == (B, D) and W.shape == (V, D) and lab.shape == (B,), \
        (x.shape, W.shape, lab.shape)

    xn = x / np.maximum(np.linalg.norm(x, axis=1, keepdims=True), EPS)

    nc = _get_nc()
    in_maps, Wn = make_in_maps(xn, W)
    res = bass_utils.run_bass_kernel_spmd(
        nc, in_maps, core_ids=list(range(NCORES)), **run_kwargs)

    S = np.zeros(B, dtype=np.float64)
    for r in res.results:
        sp = r["s_out"].reshape(P, NQ, MT).sum(axis=1)  # add the quarters
        S += sp.T.reshape(-1).astype(np.float64)

    # Host label-column correction (O(B*D), fp64 epilogue).
    cos_y = np.einsum("bd,bd->b", xn.astype(np.float64),
                      Wn[lab].astype(np.float64))
    sin_y = np.sqrt(np.clip(1.0 - cos_y * cos_y, 0.0, 1.0))
    phi_y = cos_y * math.cos(M_MARGIN) - sin_y * math.sin(M_MARGIN)
    S += np.exp(S_SCALE * phi_y - SHIFT) - np.exp(S_SCALE * cos_y - SHIFT)
    loss = np.mean(SHIFT + np.log(S) - S_SCALE * phi_y)

    kernel.last_results = res
    return np.asarray(loss, dtype=np.float32)
